# revision 2
# baseline (speedup 1.0000x reference)
"""ConcatLoRALinear on 8 trn2 NeuronCores, column-parallel, fp8 DoubleRow v4.

out = x @ W.T + b + SCALE * sum_e (x @ A_e.T) @ B_e.T

W_eff = W.T + A_cat.T @ (2 B_cat.T) folded on host (exact fp32), then the
device computes x @ W_eff via fp8-e4m3 DoubleRow matmuls (0.5 cyc/row, two
128-deep k-planes per instruction).

Precision scheme (rel err ~1.6e-2 < 2e-2 tolerance): W_eff is pre-scaled
by 64 (keeps e4m3 out of subnormals; eviction divides by 64) and split
into hi + lo fp8 planes; x is split hi + lo on 26 of the 32 k-chunks.
Kept products per split chunk: xh*wh + xh*wl + xl*wh (lo*lo dropped);
the 6 "single" chunks keep only xh*wh. That is 84 products = 42
DoubleRow instructions per [128tok x 512out] PSUM group, vs 32 bf16
matmuls: PE time 0.656x of the bf16 floor.

Products are paired across chunk pairs (c,c') so every instruction reads
two ADJACENT x planes (no stride-0, no duplicated x bytes); the tiny W
planes are laid out per-instruction with duplicates.
"""

import numpy as np
import ml_dtypes

import concourse.bass as bass  # noqa: F401  (bass must import before tile)
import concourse.mybir as mybir
import concourse.tile as tile
from concourse import bacc
from concourse.bass_utils import run_bass_kernel_spmd

F32 = mybir.dt.float32
BF16 = mybir.dt.bfloat16
FP8 = mybir.dt.float8e4
E4 = np.dtype(ml_dtypes.float8_e4m3)

SCALE = 2.0  # alpha/r = 16/8
WSCALE = 64.0  # W_eff pre-scale (power of 2; divided out at eviction)
N_CORES = 8
T = 8192  # tokens = 4*2048
D = 4096  # in_features (contraction)
O_SH = 512  # out_features per core
KC = 32  # contraction chunks of 128
N_FULL = 26  # chunks with x split hi+lo; the rest keep xh*wh only
NPX = N_FULL * 2 + (KC - N_FULL)  # 58 x planes
TS0 = 1024  # super-0 token width (8 PSUM groups)
TS = 512  # steady-state super width (4 PSUM groups)
N_SUPER = 1 + (T - TS0) // TS  # 15

_CACHE = {}


def _instr_table():
    """42 DoubleRow instructions per PSUM group.

    x planes, pair-interleaved: np 4i..4i+3 = xh(2i), xh(2i+1), xl(2i),
    xl(2i+1) for chunk pair i < 13; np 52..57 = xh(26..31).
    Returns (instrs, wplanes, xplane_src): instrs[j] = xnp0 (x planes
    xnp0, xnp0+1); wplanes = 84 entries ('h'|'l', chunk), instruction j
    reads 2j, 2j+1; xplane_src[np] = ('h'|'l', chunk).
    """
    instrs = []
    wplanes = []
    xsrc = [None] * NPX
    for i in range(N_FULL // 2):
        c, c2 = 2 * i, 2 * i + 1
        xsrc[4 * i] = ("h", c)
        xsrc[4 * i + 1] = ("h", c2)
        xsrc[4 * i + 2] = ("l", c)
        xsrc[4 * i + 3] = ("l", c2)
        instrs.append(4 * i)
        wplanes += [("h", c), ("l", c2)]
        instrs.append(4 * i)
        wplanes += [("l", c), ("h", c2)]
        instrs.append(4 * i + 2)
        wplanes += [("h", c), ("h", c2)]
    for j in range(KC - N_FULL):
        xsrc[2 * N_FULL + j] = ("h", N_FULL + j)
    for j in range((KC - N_FULL) // 2):
        instrs.append(2 * N_FULL + 2 * j)
        wplanes += [("h", N_FULL + 2 * j), ("h", N_FULL + 2 * j + 1)]
    return instrs, wplanes, xsrc


INSTRS, WPLANES, XSRC = _instr_table()
NI = len(INSTRS)  # 42
NPW = len(WPLANES)  # 84

# super-0 x granules follow instruction order (contiguous plane runs);
# first few are 2-plane for a fast pipeline fill
X0_GRANS = ([[0, 1], [2, 3], [4, 5], [6, 7]]
            + [list(range(8 + 4 * i, 12 + 4 * i)) for i in range(11)]
            + [list(range(2 * N_FULL, NPX))])
# w granules for the prologue: (start, end) plane ranges
W_GRANS = [(0, 2), (2, 4)] + [(4 + 6 * i, 10 + 6 * i) for i in range(13)] + [(82, 84)]
# steady-state granules: plain plane order, even splits
XS_GRANS = [list(range(0, 16)), list(range(16, 32)),
            list(range(32, 48)), list(range(48, NPX))]


def _gran_index(grans):
    m = {}
    for gi, planes in enumerate(grans):
        for off, np_ in enumerate(planes):
            m[np_] = (gi, off)
    return m


X0_IDX = _gran_index(X0_GRANS)
XS_IDX = _gran_index(XS_GRANS)


def _build():
    nc = bacc.Bacc("TRN2", target_bir_lowering=False, debug=False,
                   num_devices=N_CORES)

    xq_d = nc.dram_tensor("xq", [NPX * 128, T], FP8, kind="ExternalInput")
    wq_d = nc.dram_tensor("wq", [NPW * 128, O_SH], FP8, kind="ExternalInput")
    bias_d = nc.dram_tensor("bias", [128, O_SH], F32, kind="ExternalInput")
    out_d = nc.dram_tensor("out", [T, O_SH], F32, kind="ExternalOutput")

    xq_r = xq_d.ap().rearrange("(np p) t -> p np t", p=128)  # [128, NPX, T]
    wq_r = wq_d.ap().rearrange("(np p) o -> p np o", p=128)  # [128, NPW, O_SH]
    out_r = out_d.ap().rearrange("(t p) o -> p t o", p=128)

    def tok0(s):
        return 0 if s == 0 else TS0 + (s - 1) * TS

    with tile.TileContext(nc) as tc:
        with (
            tc.tile_pool(name="const", bufs=1) as const,
            tc.tile_pool(name="x0_p", bufs=len(X0_GRANS)) as x0_p,
            tc.tile_pool(name="x_p", bufs=2 * len(XS_GRANS)) as x_p,
            tc.tile_pool(name="o_p", bufs=4) as o_p,
            tc.tile_pool(name="ps_p", bufs=8, space="PSUM") as ps_p,
        ):
            wq = const.tile([128, NPW, O_SH], FP8)
            bias_sb = const.tile([128, O_SH], F32)

            # Warm the PE pstate clock: tiny dummies pin pe_busy_start
            # early, then ap-256 dummies keep the PE queue non-empty (and
            # accumulate >3us of busy) across the DMA lead-in so the first
            # real matmuls dispatch at 2.4 GHz.
            wm_sb = const.tile([128, 8], BF16)
            wm2_sb = const.tile([128, 256], BF16)
            nc.gpsimd.memset(wm_sb[:], 0.0)
            nc.gpsimd.memset(wm2_sb[:], 0.0)
            wm_ps = ps_p.tile([128, O_SH], F32, tag="ps", name="ps")
            for _ in range(24):
                nc.tensor.matmul(wm_ps[0:8, 0:1], lhsT=wm_sb[:],
                                 rhs=wm_sb[:, 0:1], start=True, stop=True)
            for _ in range(12):
                nc.tensor.matmul(wm_ps[0:8, 0:256], lhsT=wm_sb[:],
                                 rhs=wm2_sb[:], start=True, stop=True)

            # Prologue stream: per instruction-triple i, w granule (6
            # planes) then x granule (4 planes of super-0 tokens).
            x0 = []
            for gi, planes in enumerate(X0_GRANS):
                if gi < len(W_GRANS):
                    w0, w1 = W_GRANS[gi]
                    nc.sync.dma_start(out=wq[:, w0:w1, :],
                                      in_=wq_r[:, w0:w1, :])
                xg = x0_p.tile([128, len(planes), TS0], FP8, tag="x0",
                               name="x0g")
                nc.sync.dma_start(out=xg[:], in_=xq_r[:, planes[0]:
                                                      planes[-1] + 1, 0:TS0])
                x0.append(xg)
                if gi == 1:
                    nc.sync.dma_start(out=bias_sb[:], in_=bias_d.ap())

            xs_all = {0: x0}

            def emit_steady(s):
                gl = []
                for planes in XS_GRANS:
                    xg = x_p.tile([128, len(planes), TS], FP8, tag="xt",
                                  name="xg")
                    nc.sync.dma_start(
                        out=xg[:],
                        in_=xq_r[:, planes[0]:planes[-1] + 1,
                                 tok0(s):tok0(s) + TS],
                    )
                    gl.append(xg)
                xs_all[s] = gl

            def x_lhsT(s, j, ti):
                np0 = INSTRS[j]
                gi, off = (X0_IDX if s == 0 else XS_IDX)[np0]
                return xs_all[s][gi][:, off:off + 2,
                                     ti * 128:(ti + 1) * 128]

            def evict(s, ti, ps):
                # bias_sb holds 64*b; host divides the gathered out by 64
                ot = o_p.tile([128, O_SH], F32, tag="ot", name="ot")
                nc.vector.tensor_tensor(ot[:], ps[:], bias_sb[:],
                                        op=mybir.AluOpType.add)
                nc.sync.dma_start(
                    out=out_r[:, tok0(s) // 128 + ti, :], in_=ot[:])

            def mm(ps, s, j, ti):
                nc.tensor.matmul(
                    ps[:],
                    lhsT=x_lhsT(s, j, ti),
                    rhs=wq[:, 2 * j:2 * j + 2, :],
                    start=(j == 0),
                    stop=(j == NI - 1),
                    perf_mode=mybir.MatmulPerfMode.DoubleRow,
                )

            emit_steady(1)

            for s in range(N_SUPER):
                ngroups = (TS0 if s == 0 else TS) // 128
                if s == 0:
                    # instruction-outer: each granule pair unlocks
                    # 8 groups x 3 instructions of PE work
                    pss = [ps_p.tile([128, O_SH], F32, tag="ps", name="ps")
                           for _ in range(ngroups)]
                    for j in range(NI):
                        for ti in range(ngroups):
                            mm(pss[ti], s, j, ti)
                    for ti in range(ngroups):
                        evict(s, ti, pss[ti])
                else:
                    for ti in range(ngroups):
                        if ti == 0 and s + 1 < N_SUPER:
                            emit_steady(s + 1)
                        if s == N_SUPER - 1 and ti == ngroups - 1:
                            # strip-split the final group so its eviction
                            # pipelines with the remaining matmuls; separate
                            # PSUM tiles avoid false inter-strip deps
                            for st in range(4):
                                osl = slice(st * 128, (st + 1) * 128)
                                pst = ps_p.tile([128, O_SH], F32, tag="ps",
                                                name="ps")
                                for j in range(NI):
                                    nc.tensor.matmul(
                                        pst[:, 0:128],
                                        lhsT=x_lhsT(s, j, ti),
                                        rhs=wq[:, 2 * j:2 * j + 2, osl],
                                        start=(j == 0),
                                        stop=(j == NI - 1),
                                        perf_mode=mybir.MatmulPerfMode.DoubleRow,
                                    )
                                ots = o_p.tile([128, 128], F32, tag="ots",
                                               name="ots")
                                nc.vector.tensor_tensor(
                                    ots[:], pst[:, 0:128], bias_sb[:, osl],
                                    op=mybir.AluOpType.add)
                                nc.sync.dma_start(
                                    out=out_r[:, tok0(s) // 128 + ti, osl],
                                    in_=ots[:])
                        else:
                            ps = ps_p.tile([128, O_SH], F32, tag="ps",
                                           name="ps")
                            for j in range(NI):
                                mm(ps, s, j, ti)
                            evict(s, ti, ps)
    nc.compile()
    return nc


def _quant_planes(x, W, b, A, B):
    """Host-side quantization. Returns (xq shared, per-core wq list, bias)."""
    a_cat = A.reshape(8 * 8, D)
    b_cat = (B * SCALE).transpose(0, 2, 1).reshape(8 * 8, D)
    weff = (W.T + a_cat.T @ b_cat) * WSCALE  # [D, O_total] fp32, pre-scaled
    xt = np.ascontiguousarray(x.reshape(T, D).T)  # [D, T]

    xq = np.empty((NPX, 128, T), dtype=E4)
    xh_cache = {}
    for c in range(KC):
        ks = slice(c * 128, (c + 1) * 128)
        xh_cache[c] = xt[ks].astype(E4)
    for np_, (kind, c) in enumerate(XSRC):
        ks = slice(c * 128, (c + 1) * 128)
        if kind == "h":
            xq[np_] = xh_cache[c]
        else:
            xq[np_] = (xt[ks] - xh_cache[c].astype(np.float32)).astype(E4)
    xq = xq.reshape(NPX * 128, T)

    wh = np.empty((KC, 128, 4096), dtype=E4)
    wl = np.empty((KC, 128, 4096), dtype=E4)
    for c in range(KC):
        ks = slice(c * 128, (c + 1) * 128)
        h = weff[ks].astype(E4)
        wh[c] = h
        wl[c] = (weff[ks] - h.astype(np.float32)).astype(E4)
    return xq, wh, wl


def _shards(x, W, b, A, B):
    xq, wh, wl = _quant_planes(x, W, b, A, B)
    in_maps = []
    for c in range(N_CORES):
        sl = slice(c * O_SH, (c + 1) * O_SH)
        wq = np.empty((NPW, 128, O_SH), dtype=E4)
        for j, (kind, ch) in enumerate(WPLANES):
            src = wh if kind == "h" else wl
            wq[j] = src[ch][:, sl]
        in_maps.append({
            "xq": xq,
            "wq": wq.reshape(NPW * 128, O_SH),
            "bias": np.ascontiguousarray(
                np.broadcast_to((WSCALE * b[sl])[None, :],
                                (128, O_SH)).astype(np.float32)
            ),
        })
    return in_maps


def kernel(x, W, b, A, B):
    x = np.asarray(x, dtype=np.float32)
    W = np.asarray(W, dtype=np.float32)
    b = np.asarray(b, dtype=np.float32)
    A = np.asarray(A, dtype=np.float32)
    B = np.asarray(B, dtype=np.float32)

    if "nc" not in _CACHE:
        _CACHE["nc"] = _build()
    nc = _CACHE["nc"]

    in_maps = _shards(x, W, b, A, B)
    res = run_bass_kernel_spmd(nc, in_maps, core_ids=list(range(N_CORES)))
    out = np.concatenate([res.results[c]["out"] for c in range(N_CORES)], axis=1)
    out *= (1.0 / WSCALE)
    return out.reshape(4, 2048, 4096)


# revision 3
# speedup vs baseline: 1.0003x; 1.0003x over previous
"""ConcatLoRALinear on 8 trn2 NeuronCores, column-parallel, fp8 DoubleRow v4.

out = x @ W.T + b + SCALE * sum_e (x @ A_e.T) @ B_e.T

W_eff = W.T + A_cat.T @ (2 B_cat.T) folded on host (exact fp32), then the
device computes x @ W_eff via fp8-e4m3 DoubleRow matmuls (0.5 cyc/row, two
128-deep k-planes per instruction).

Precision scheme (rel err ~1.6e-2 < 2e-2 tolerance): W_eff is pre-scaled
by 64 (keeps e4m3 out of subnormals; eviction divides by 64) and split
into hi + lo fp8 planes; x is split hi + lo on 26 of the 32 k-chunks.
Kept products per split chunk: xh*wh + xh*wl + xl*wh (lo*lo dropped);
the 6 "single" chunks keep only xh*wh. That is 84 products = 42
DoubleRow instructions per [128tok x 512out] PSUM group, vs 32 bf16
matmuls: PE time 0.656x of the bf16 floor.

Products are paired across chunk pairs (c,c') so every instruction reads
two ADJACENT x planes (no stride-0, no duplicated x bytes); the tiny W
planes are laid out per-instruction with duplicates.
"""

import numpy as np
import ml_dtypes

import concourse.bass as bass  # noqa: F401  (bass must import before tile)
import concourse.mybir as mybir
import concourse.tile as tile
from concourse import bacc
from concourse.bass_utils import run_bass_kernel_spmd

F32 = mybir.dt.float32
BF16 = mybir.dt.bfloat16
FP8 = mybir.dt.float8e4
E4 = np.dtype(ml_dtypes.float8_e4m3)

SCALE = 2.0  # alpha/r = 16/8
WSCALE = 64.0  # W_eff pre-scale (power of 2; divided out at eviction)
N_CORES = 8
T = 8192  # tokens = 4*2048
D = 4096  # in_features (contraction)
O_SH = 512  # out_features per core
KC = 32  # contraction chunks of 128
N_FULL = 26  # chunks with x split hi+lo; the rest keep xh*wh only
NPX = N_FULL * 2 + (KC - N_FULL)  # 58 x planes
TS0 = 1024  # super-0 token width (8 PSUM groups)
TS = 512  # steady-state super width (4 PSUM groups)
N_SUPER = 1 + (T - TS0) // TS  # 15

_CACHE = {}


def _instr_table():
    """42 DoubleRow instructions per PSUM group.

    x planes, pair-interleaved: np 4i..4i+3 = xh(2i), xh(2i+1), xl(2i),
    xl(2i+1) for chunk pair i < 13; np 52..57 = xh(26..31).
    Returns (instrs, wplanes, xplane_src): instrs[j] = xnp0 (x planes
    xnp0, xnp0+1); wplanes = 84 entries ('h'|'l', chunk), instruction j
    reads 2j, 2j+1; xplane_src[np] = ('h'|'l', chunk).
    """
    instrs = []       # xnp0 per instruction
    wslices = []      # (start, stop, step) w-plane slice per instruction
    # w planes, deduped: wh_c at 2c, wl_c at 2c+1 (c < N_FULL);
    # wh_{N_FULL+j} at 2*N_FULL + j
    wsrc = [None] * (2 * N_FULL + (KC - N_FULL))
    xsrc = [None] * NPX
    for i in range(N_FULL // 2):
        c, c2 = 2 * i, 2 * i + 1
        xsrc[4 * i] = ("h", c)
        xsrc[4 * i + 1] = ("h", c2)
        xsrc[4 * i + 2] = ("l", c)
        xsrc[4 * i + 3] = ("l", c2)
        wsrc[4 * i] = ("h", c)
        wsrc[4 * i + 1] = ("l", c)
        wsrc[4 * i + 2] = ("h", c2)
        wsrc[4 * i + 3] = ("l", c2)
        instrs.append(4 * i)
        wslices.append((4 * i, 4 * i + 4, 3))      # (wh_c, wl_c2)
        instrs.append(4 * i)
        wslices.append((4 * i + 1, 4 * i + 3, 1))  # (wl_c, wh_c2)
        instrs.append(4 * i + 2)
        wslices.append((4 * i, 4 * i + 3, 2))      # (wh_c, wh_c2)
    for j in range(KC - N_FULL):
        xsrc[2 * N_FULL + j] = ("h", N_FULL + j)
        wsrc[2 * N_FULL + j] = ("h", N_FULL + j)
    for j in range((KC - N_FULL) // 2):
        instrs.append(2 * N_FULL + 2 * j)
        wslices.append((2 * N_FULL + 2 * j, 2 * N_FULL + 2 * j + 2, 1))
    return instrs, wslices, xsrc, wsrc


INSTRS, WSLICES, XSRC, WSRC = _instr_table()
NI = len(INSTRS)  # 42
NPW = len(WSRC)  # 58

# super-0 x granules follow instruction order (contiguous plane runs);
# first few are 2-plane for a fast pipeline fill
X0_GRANS = ([[0, 1], [2, 3], [4, 5], [6, 7]]
            + [list(range(8 + 4 * i, 12 + 4 * i)) for i in range(11)]
            + [list(range(2 * N_FULL, NPX))])
# w granules for the prologue, aligned 1:1 with instruction triples
W_GRANS = [(4 * i, 4 * i + 4) for i in range(N_FULL // 2)] + [
    (2 * N_FULL, NPW)]
# steady-state granules: plain plane order, even splits
XS_GRANS = [list(range(0, 16)), list(range(16, 32)),
            list(range(32, 48)), list(range(48, NPX))]


def _gran_index(grans):
    m = {}
    for gi, planes in enumerate(grans):
        for off, np_ in enumerate(planes):
            m[np_] = (gi, off)
    return m


X0_IDX = _gran_index(X0_GRANS)
XS_IDX = _gran_index(XS_GRANS)


def _build():
    nc = bacc.Bacc("TRN2", target_bir_lowering=False, debug=False,
                   num_devices=N_CORES)

    xq_d = nc.dram_tensor("xq", [NPX * 128, T], FP8, kind="ExternalInput")
    wq_d = nc.dram_tensor("wq", [NPW * 128, O_SH], FP8, kind="ExternalInput")
    bias_d = nc.dram_tensor("bias", [128, O_SH], F32, kind="ExternalInput")
    out_d = nc.dram_tensor("out", [T, O_SH], F32, kind="ExternalOutput")

    xq_r = xq_d.ap().rearrange("(np p) t -> p np t", p=128)  # [128, NPX, T]
    wq_r = wq_d.ap().rearrange("(np p) o -> p np o", p=128)  # [128, NPW, O_SH]
    out_r = out_d.ap().rearrange("(t p) o -> p t o", p=128)

    def tok0(s):
        return 0 if s == 0 else TS0 + (s - 1) * TS

    with tile.TileContext(nc) as tc:
        with (
            tc.tile_pool(name="const", bufs=1) as const,
            tc.tile_pool(name="x0_p", bufs=len(X0_GRANS)) as x0_p,
            tc.tile_pool(name="x_p", bufs=2 * len(XS_GRANS)) as x_p,
            tc.tile_pool(name="o_p", bufs=4) as o_p,
            tc.tile_pool(name="ps_p", bufs=8, space="PSUM") as ps_p,
        ):
            wq = const.tile([128, NPW, O_SH], FP8)
            bias_sb = const.tile([128, O_SH], F32)

            # Warm the PE pstate clock: tiny dummies pin pe_busy_start
            # early, then ap-256 dummies keep the PE queue non-empty (and
            # accumulate >3us of busy) across the DMA lead-in so the first
            # real matmuls dispatch at 2.4 GHz.
            wm_sb = const.tile([128, 8], BF16)
            wm2_sb = const.tile([128, 256], BF16)
            nc.gpsimd.memset(wm_sb[:], 0.0)
            nc.gpsimd.memset(wm2_sb[:], 0.0)
            wm_ps = ps_p.tile([128, O_SH], F32, tag="ps", name="ps")
            for _ in range(24):
                nc.tensor.matmul(wm_ps[0:8, 0:1], lhsT=wm_sb[:],
                                 rhs=wm_sb[:, 0:1], start=True, stop=True)
            for _ in range(12):
                nc.tensor.matmul(wm_ps[0:8, 0:256], lhsT=wm_sb[:],
                                 rhs=wm2_sb[:], start=True, stop=True)

            # Prologue stream: per instruction-triple i, w granule (6
            # planes) then x granule (4 planes of super-0 tokens).
            x0 = []
            for gi, planes in enumerate(X0_GRANS):
                if gi < len(W_GRANS):
                    w0, w1 = W_GRANS[gi]
                    nc.sync.dma_start(out=wq[:, w0:w1, :],
                                      in_=wq_r[:, w0:w1, :])
                xg = x0_p.tile([128, len(planes), TS0], FP8, tag="x0",
                               name="x0g")
                nc.sync.dma_start(out=xg[:], in_=xq_r[:, planes[0]:
                                                      planes[-1] + 1, 0:TS0])
                x0.append(xg)
                if gi == 1:
                    nc.sync.dma_start(out=bias_sb[:], in_=bias_d.ap())

            xs_all = {0: x0}

            def emit_steady(s):
                gl = []
                for planes in XS_GRANS:
                    xg = x_p.tile([128, len(planes), TS], FP8, tag="xt",
                                  name="xg")
                    nc.sync.dma_start(
                        out=xg[:],
                        in_=xq_r[:, planes[0]:planes[-1] + 1,
                                 tok0(s):tok0(s) + TS],
                    )
                    gl.append(xg)
                xs_all[s] = gl

            def x_lhsT(s, j, ti):
                np0 = INSTRS[j]
                gi, off = (X0_IDX if s == 0 else XS_IDX)[np0]
                return xs_all[s][gi][:, off:off + 2,
                                     ti * 128:(ti + 1) * 128]

            def evict(s, ti, ps):
                # bias_sb holds 64*b; host divides the gathered out by 64
                ot = o_p.tile([128, O_SH], F32, tag="ot", name="ot")
                nc.vector.tensor_tensor(ot[:], ps[:], bias_sb[:],
                                        op=mybir.AluOpType.add)
                nc.sync.dma_start(
                    out=out_r[:, tok0(s) // 128 + ti, :], in_=ot[:])

            def mm(ps, s, j, ti):
                a, b, st = WSLICES[j]
                nc.tensor.matmul(
                    ps[:],
                    lhsT=x_lhsT(s, j, ti),
                    rhs=wq[:, a:b:st, :],
                    start=(j == 0),
                    stop=(j == NI - 1),
                    perf_mode=mybir.MatmulPerfMode.DoubleRow,
                )

            emit_steady(1)

            for s in range(N_SUPER):
                ngroups = (TS0 if s == 0 else TS) // 128
                if s == 0:
                    # instruction-outer: each granule pair unlocks
                    # 8 groups x 3 instructions of PE work
                    pss = [ps_p.tile([128, O_SH], F32, tag="ps", name="ps")
                           for _ in range(ngroups)]
                    for j in range(NI):
                        for ti in range(ngroups):
                            mm(pss[ti], s, j, ti)
                    for ti in range(ngroups):
                        evict(s, ti, pss[ti])
                else:
                    for ti in range(ngroups):
                        if ti == 0 and s + 1 < N_SUPER:
                            emit_steady(s + 1)
                        if s == N_SUPER - 1 and ti == ngroups - 1:
                            # strip-split the final group so its eviction
                            # pipelines with the remaining matmuls; separate
                            # PSUM tiles avoid false inter-strip deps
                            for st in range(4):
                                osl = slice(st * 128, (st + 1) * 128)
                                pst = ps_p.tile([128, O_SH], F32, tag="ps",
                                                name="ps")
                                for j in range(NI):
                                    a, b, st = WSLICES[j]
                                    nc.tensor.matmul(
                                        pst[:, 0:128],
                                        lhsT=x_lhsT(s, j, ti),
                                        rhs=wq[:, a:b:st, osl],
                                        start=(j == 0),
                                        stop=(j == NI - 1),
                                        perf_mode=mybir.MatmulPerfMode.DoubleRow,
                                    )
                                ots = o_p.tile([128, 128], F32, tag="ots",
                                               name="ots")
                                nc.vector.tensor_tensor(
                                    ots[:], pst[:, 0:128], bias_sb[:, osl],
                                    op=mybir.AluOpType.add)
                                nc.sync.dma_start(
                                    out=out_r[:, tok0(s) // 128 + ti, osl],
                                    in_=ots[:])
                        else:
                            ps = ps_p.tile([128, O_SH], F32, tag="ps",
                                           name="ps")
                            for j in range(NI):
                                mm(ps, s, j, ti)
                            evict(s, ti, ps)
    nc.compile()
    return nc


def _quant_planes(x, W, b, A, B):
    """Host-side quantization. Returns (xq shared, per-core wq list, bias)."""
    a_cat = A.reshape(8 * 8, D)
    b_cat = (B * SCALE).transpose(0, 2, 1).reshape(8 * 8, D)
    weff = (W.T + a_cat.T @ b_cat) * WSCALE  # [D, O_total] fp32, pre-scaled
    xt = np.ascontiguousarray(x.reshape(T, D).T)  # [D, T]

    xq = np.empty((NPX, 128, T), dtype=E4)
    xh_cache = {}
    for c in range(KC):
        ks = slice(c * 128, (c + 1) * 128)
        xh_cache[c] = xt[ks].astype(E4)
    for np_, (kind, c) in enumerate(XSRC):
        ks = slice(c * 128, (c + 1) * 128)
        if kind == "h":
            xq[np_] = xh_cache[c]
        else:
            xq[np_] = (xt[ks] - xh_cache[c].astype(np.float32)).astype(E4)
    xq = xq.reshape(NPX * 128, T)

    wh = np.empty((KC, 128, 4096), dtype=E4)
    wl = np.empty((KC, 128, 4096), dtype=E4)
    for c in range(KC):
        ks = slice(c * 128, (c + 1) * 128)
        h = weff[ks].astype(E4)
        wh[c] = h
        wl[c] = (weff[ks] - h.astype(np.float32)).astype(E4)
    return xq, wh, wl


def _shards(x, W, b, A, B):
    xq, wh, wl = _quant_planes(x, W, b, A, B)
    in_maps = []
    for c in range(N_CORES):
        sl = slice(c * O_SH, (c + 1) * O_SH)
        wq = np.empty((NPW, 128, O_SH), dtype=E4)
        for j, (kind, ch) in enumerate(WSRC):
            src = wh if kind == "h" else wl
            wq[j] = src[ch][:, sl]
        in_maps.append({
            "xq": xq,
            "wq": wq.reshape(NPW * 128, O_SH),
            "bias": np.ascontiguousarray(
                np.broadcast_to((WSCALE * b[sl])[None, :],
                                (128, O_SH)).astype(np.float32)
            ),
        })
    return in_maps


def kernel(x, W, b, A, B):
    x = np.asarray(x, dtype=np.float32)
    W = np.asarray(W, dtype=np.float32)
    b = np.asarray(b, dtype=np.float32)
    A = np.asarray(A, dtype=np.float32)
    B = np.asarray(B, dtype=np.float32)

    if "nc" not in _CACHE:
        _CACHE["nc"] = _build()
    nc = _CACHE["nc"]

    in_maps = _shards(x, W, b, A, B)
    res = run_bass_kernel_spmd(nc, in_maps, core_ids=list(range(N_CORES)))
    out = np.concatenate([res.results[c]["out"] for c in range(N_CORES)], axis=1)
    out *= (1.0 / WSCALE)
    return out.reshape(4, 2048, 4096)
